# revision 12
# baseline (speedup 1.0000x reference)
"""TRN2 Bass kernel for nn_EvolutionModel_91173565759692 (self-contained).

Physics: 16384 rays, 100-step velocity-Verlet in ior-center-centered coords
  y_{t+1} = W(g)*y_t - y_{t-1},  W = (g*c2 + c1)*g + 2,  g = exp(-2|y|^2)
exp computed as g ~= (1 - alpha*r2)^128 via repeated squaring, which fuses
the whole step into 4 DVE ops (SQ2S, V16, QW, sub).
Sampling: bucket LUT (width 2^-6) searchsorted, fp16 payload channels, no
correction round (off-by-one brackets are collinear => error ~1e-3 << tol).
8-way data-parallel over rays (2048 rays/core).
"""
import sys
sys.path.insert(0, "/opt/trn_rl_repo")
import numpy as np
import concourse.bass as bass
import concourse.bacc as bacc
import concourse.mybir as mybir
from concourse.tile import TileContext
import concourse.dve_ops as dve_ops
from concourse import dve_spec
from concourse.dve_spec import Spec, Src0, Src1, C0, C1, C2, One, sq, lower
from concourse.dve_uop import DveOpSpec
from concourse.dve_table_gen import dve_ver_for
from concourse.bass_utils import run_bass_kernel_spmd

f32 = mybir.dt.float32
f16 = mybir.dt.float16
i16 = mybir.dt.int16
AF = mybir.ActivationFunctionType
ALU = mybir.AluOpType

N_STEPS = 100
DT = np.float32(0.02)
KC = np.float32(-DT * DT / np.float32(0.25))   # -dt^2/sigma^2
ALPHA = float(np.float32((2.0 / 128.0) * (1.0 - 1.0 / 128.0)))

_registered = {}


def register_op(name, spec, subdim=False):
    if name in _registered:
        return _registered[name]
    ver = dve_ver_for("TRN2")
    row = dve_ops._CUSTOM_DVE_ROW_BASE + len(dve_ops.OPS)
    assert row < 0x20
    dve_ops._SUB_OPCODE_FOR_NAME[name] = row
    tmp = DveOpSpec(name=name, opcode=row, uops=lower(spec, ver=ver),
                    rd1_en=dve_spec._has_src1(spec))
    op = dve_ops.DveOp(name, spec, subdim, {ver: tmp.sha(ver)})
    dve_ops.OPS.append(op)
    dve_ops.CUSTOM_DVE_SPECS[name] = spec
    _registered[name] = op
    return op


# t12 = (y0^2 + y1^2) * alpha
OP_SQ2S = lambda: register_op(
    "ANT2_SQ2S",
    Spec(body=(sq(Src0) + sq(Src1)) * C0,
         reference=lambda in0, in1, s0, s1, imm2: (
             (in0.astype(np.float32) ** 2 + in1.astype(np.float32) ** 2) * s0)))


# v16 = (1 - (alpha*y2^2 + t12))^16
def _v16_ref(in0, in1, s0, s1, imm2):
    v = 1.0 - (in0.astype(np.float32) ** 2 * s0 + in1.astype(np.float32))
    for _ in range(4):
        v = v * v
    return v


def _v16_body():
    v = One - (sq(Src0) * C0 + Src1)
    for _ in range(4):
        v = sq(v)
    return v


OP_V16 = lambda: register_op("ANT2_V16", Spec(body=_v16_body(), reference=_v16_ref))


# q = y * ((g*c2 + c1)*g + k), g = Src1^8 (Src1 = v16 bcast), k = 2 or 1
def _qwk_body(k):
    g = sq(sq(sq(Src1)))
    w = (g * C0 + C1) * g + (One + One if k == 2 else One)
    return Src0 * w


def _qwk_ref(k):
    def ref(in0, in1, s0, s1, imm2):
        g = in1.astype(np.float32)
        for _ in range(3):
            g = g * g
        return in0.astype(np.float32) * ((g * s0 + s1) * g + float(k))
    return ref


OP_QW2N = lambda: register_op("ANT2_QW2", Spec(body=_qwk_body(2), reference=_qwk_ref(2)))
OP_QW1N = lambda: register_op("ANT2_QW1", Spec(body=_qwk_body(1), reference=_qwk_ref(1)))

# out = Src0*Src0 + Src1*Src1
OP_SQ2 = lambda: register_op(
    "ANT_EVO_SQ2",
    Spec(body=Src0 * Src0 + Src1 * Src1,
         reference=lambda in0, in1, s0, s1, imm2: (
             in0.astype(np.float32) ** 2 + in1.astype(np.float32) ** 2)))

# out = Src0*Src0 + Src1
OP_SQA = lambda: register_op(
    "ANT_EVO_SQA",
    Spec(body=Src0 * Src0 + Src1,
         reference=lambda in0, in1, s0, s1, imm2: (
             in0.astype(np.float32) ** 2 + in1.astype(np.float32))))

# out = Src0*Src1 - One
OP_MUL_SUB1 = lambda: register_op(
    "ANT_EVO_MULSUB1",
    Spec(body=Src0 * Src1 - One,
         reference=lambda in0, in1, s0, s1, imm2: (
             in0.astype(np.float32) * in1 - 1.0)))

# out = (Src0*C0 + C1) + Src1
OP_AFF2 = lambda: register_op(
    "ANT_EVO_AFF2",
    Spec(body=(Src0 * C0 + C1) + Src1,
         reference=lambda in0, in1, s0, s1, imm2: (
             in0.astype(np.float32) * s0 + s1) + in1))


BUCK = 124          # buckets per ray (width 2^-6; bt clamped at 123)
BSP = 16 * BUCK     # 1984
TS = 102            # T-slots per ray (101 steps + pad)
NTS = 16 * TS       # 1632

CONST_SPECS = (("gvals", "i16", NTS), ("cboffT", "f32", NTS),
               ("cboffZ1", "f16", 1024), ("sglob1", "i16", 1024),
               ("cfold2", "f32", 1024), ("cwrap", "f32", 1024))
_DTMAP = {"i16": i16, "f32": f32, "f16": f16}


def host_consts():
    """Constant helper tensors (tiled to 128 partitions)."""
    j = np.arange(16, dtype=np.int64)[:, None]
    t = np.arange(TS, dtype=np.int64)[None, :]
    s64 = np.arange(64, dtype=np.int64)[None, :]
    out = {}
    gv = (j * 128 + t + 1).astype(np.int16)
    gv[:, 101] = 0
    out["gvals"] = gv.reshape(-1)
    cb = (j * BUCK - 0.499 + 0 * t).astype(np.float32)
    cb[:, 101] = -10000.0
    out["cboffT"] = cb.reshape(-1)
    out["cboffZ1"] = (j * BUCK + 1.0 + 0 * s64).astype(np.float16).reshape(-1)
    out["sglob1"] = (j * 64 + s64 + 1).astype(np.int16).reshape(-1)
    out["cfold2"] = (j * TS - j * 128 + 0 * s64).astype(np.float32).reshape(-1)
    out["cwrap"] = (j * TS + 100.5 + 0 * s64).astype(np.float32).reshape(-1)
    return {k: np.tile(v[None, :], (128, 1)).copy() for k, v in out.items()}


def build_integration(nc, tc, pool, x0c, v0c, A, cvec, H):
    """100-step loop -> H [128,101,48] f32 SBUF."""
    v = nc.vector
    sq2s = OP_SQ2S()
    v16op = OP_V16()
    qw2 = OP_QW2N()
    qw1 = OP_QW1N()

    A = float(np.float32(A))
    c1f = float(np.float32(KC) * np.float32(A))
    c2f = float(np.float32(c1f) * np.float32(A))
    c1hf = float(np.float32(c1f) * np.float32(0.5))
    c2hf = float(np.float32(c2f) * np.float32(0.5))

    x0t = pool.tile([128, 48], f32)
    nc.sync.dma_start(x0t[:, :], x0c[:, :])
    u0 = pool.tile([128, 48], f32)
    nc.sync.dma_start(u0[:, :], v0c[:, :])

    H3 = H  # [128, 101, 48]
    v.tensor_scalar_mul(u0[:, :], u0[:, :], float(DT))  # u0 = dt*v0
    x03 = x0t[:, :].rearrange("p (a c) -> p a c", c=3)
    h03 = H3[:, 0, :].rearrange("p (a c) -> p a c", c=3)
    for ci in range(3):
        v.tensor_scalar_add(h03[:, :, ci], x03[:, :, ci],
                            -float(np.float32(cvec[ci])))

    # Two independent ray-groups (8 rays each) interleaved per step so the
    # per-dependency semaphore latency of one group hides behind the other.
    GR, gs = 2, 8
    t12 = [pool.tile([128, gs], f32, name=f"t12_{gi}") for gi in range(GR)]
    v16t = [pool.tile([128, gs], f32, name=f"v16t_{gi}") for gi in range(GR)]
    q = [pool.tile([128, gs * 3], f32, name=f"q_{gi}") for gi in range(GR)]

    def yv(t, gi):  # [128, gs, 3] view of hist at step t, group gi
        return H3[:, t, gi * gs * 3:(gi + 1) * gs * 3].rearrange(
            "p (a c) -> p a c", c=3)

    gsl = lambda gi: slice(gi * gs * 3, (gi + 1) * gs * 3)

    def step(t, op, c1x, c2x):
        for gi in range(GR):
            y3 = yv(t, gi)
            v._custom_dve(sq2s, out=t12[gi][:, :], in0=y3[:, :, 0],
                          in1=y3[:, :, 1], s0=ALPHA)
        for gi in range(GR):
            y3 = yv(t, gi)
            v._custom_dve(v16op, out=v16t[gi][:, :], in0=y3[:, :, 2],
                          in1=t12[gi][:, :], s0=ALPHA)
        for gi in range(GR):
            gb = v16t[gi][:, :].rearrange("p (a o) -> p a o", o=1).to_broadcast(
                [128, gs, 3])
            v._custom_dve(op, out=q[gi][:, :].rearrange("p (a c) -> p a c", c=3),
                          in0=yv(t, gi), in1=gb, s0=c2x, s1=c1x)

    step(0, qw1, c1hf, c2hf)                         # y1 = W1*y0 + dt*v0
    for gi in range(GR):
        v.tensor_tensor(H3[:, 1, gsl(gi)], q[gi][:, :], u0[:, gsl(gi)], ALU.add)
    for t in range(1, N_STEPS):                      # y_{t+1} = W*y_t - y_{t-1}
        step(t, qw2, c1f, c2f)
        for gi in range(GR):
            v.tensor_tensor(H3[:, t + 1, gsl(gi)], q[gi][:, :],
                            H3[:, t - 1, gsl(gi)], ALU.subtract)


def build_sampling(nc, tc, pool, H, zc, consts_dram, cvec, out_dram):
    """H: [128,101,48] SBUF f32; zc: DRAM [128,1024]; out_dram [128,3072]."""
    v = nc.vector
    s = nc.scalar
    g = nc.gpsimd
    sq2 = OP_SQ2()
    sqa = OP_SQA()
    msub1 = OP_MUL_SUB1()
    aff = OP_AFF2()

    zt = pool.tile([128, 1024], f32)
    nc.sync.dma_start(zt[:, :], zc[:, :])
    C = {}
    for name, dt_, n in CONST_SPECS:
        C[name] = pool.tile([128, n], _DTMAP[dt_], name="c_" + name)
        nc.sync.dma_start(C[name][:, :], consts_dram[name][:, :])

    # ====== z-side chain: all on Pool/ACT, overlaps the integration loop ====
    nkZ = pool.tile([128, 1024], f32, name="nkZ")
    U = pool.tile([128, BSP], i16, name="U")
    zctx = tc.tile_pool(name="zscr", bufs=1)
    zp = zctx.__enter__()
    bzr = zp.tile([128, 1024], i16, name="bzr")
    v.tensor_scalar(bzr[:, :], zt[:, :], 64.0, scalar2=-0.499,
                    op0=ALU.mult, op1=ALU.add)          # round -> floor(z*64)
    bzh = zp.tile([128, 1024], f16, name="bzh")
    v.tensor_copy(bzh[:, :], bzr[:, :])
    posZ1 = zp.tile([128, 1024], f16, name="posZ1")
    v.tensor_tensor(posZ1[:, :], bzh[:, :], C["cboffZ1"][:, :], ALU.add)
    kpZ = zp.tile([128, 16, 64], f32, name="kpZ")
    bz3 = bzr[:, :].rearrange("p (a b) -> p a b", b=64)
    v.tensor_tensor(kpZ[:, :, 0:63], bz3[:, :, 1:64], bz3[:, :, 0:63], ALU.is_gt)
    v.memset(kpZ[:, :, 63:64], 1.0)
    kpZf = kpZ[:, :, :].rearrange("p a b -> p (a b)")
    s.activation(nkZ[:, :], kpZf, AF.Copy, bias=1.0, scale=-1.0)
    idxZ = zp.tile([128, 1024], i16, name="idxZ")
    v._custom_dve(msub1, out=idxZ[:, :], in0=kpZf, in1=posZ1[:, :])
    g.local_scatter(U[:, :], C["sglob1"][:, :], idxZ[:, :],
                    channels=128, num_elems=BSP, num_idxs=1024)
    v.tensor_scalar_add(U[:, :], U[:, :], -1.0)          # U-1 (in-place)
    zctx.__exit__(None, None, None)

    # ================= T-side prep (after integration) ======================
    H3f = H[:, :, :].rearrange("p a (j c) -> p a j c", c=3)
    lctx = tc.tile_pool(name="lutscr", bufs=1)
    lp = lctx.__enter__()
    dch = []
    for ci in range(3):
        t_ = pool.tile([128, 16, TS], f16, name=f"dch{ci}")
        v.memset(t_[:, :, 100:102], 0.0)
        v.tensor_tensor(t_[:, :, 0:100].rearrange("p a b -> p b a"),
                        H3f[:, 1:101, :, ci], H3f[:, 0:100, :, ci], ALU.subtract)
        v.tensor_copy(t_[:, :, 100:101], t_[:, :, 99:100])   # dup-last-delta
        dch.append(t_)
    t2 = lp.tile([128, 16, 100], f32, name="t2scr")
    v._custom_dve(sq2, out=t2[:, :, :],
                  in0=dch[0][:, :, 0:100], in1=dch[1][:, :, 0:100])
    d2e = pool.tile([128, 16, TS], f32, name="d2e")
    v.memset(d2e[:, :, 0:1], 0.0)
    v.memset(d2e[:, :, 101:102], 0.0)
    v._custom_dve(sqa, out=d2e[:, :, 1:101],
                  in0=dch[2][:, :, 0:100], in1=t2[:, :, :])
    s.activation(d2e[:, :, 1:101], d2e[:, :, 1:101], AF.Sqrt)
    mks = lp.tile([128, 16, TS], f32, name="mks")
    v.memset(mks[:, :, :], 1.0)
    v.memset(mks[:, :, 0:1], 0.0)
    d2f = d2e[:, :, :].rearrange("p a b -> p (a b)")
    v.tensor_tensor_scan(d2f, mks[:, :, :].rearrange("p a b -> p (a b)"),
                         d2f, 0.0, ALU.mult, ALU.add)    # in-place cumsum -> D
    Dflat = d2f
    # fp16 payload channels: D and y (on ACT, off critical path)
    Dch = pool.tile([128, 16, TS], f16, name="Dch")
    s.activation(Dch[:, :, :].rearrange("p a b -> p (a b)"), Dflat, AF.Copy)
    ych = []
    for ci in range(3):
        t_ = pool.tile([128, 16, TS], f16, name=f"ych{ci}")
        v.memset(t_[:, :, 101:102], 0.0)
        src = H[:, :, :].rearrange("p a (j c) -> p j a c", c=3)[:, :, :, ci]
        s.activation(t_[:, :, 0:101], src, AF.Copy)
        ych.append(t_)

    # ================= bucket LUT -> cnt0 -> key1p ==========================
    btm = lp.tile([128, NTS], f32, name="btm")
    v.tensor_scalar(btm[:, :], Dflat, 64.0, scalar2=123.3, op0=ALU.mult, op1=ALU.min)
    posT = lp.tile([128, NTS], i16, name="posT")
    v.tensor_tensor(posT[:, :], btm[:, :], C["cboffT"][:, :], ALU.add)
    Gar = lp.tile([128, BSP], i16, name="Gar")
    g.local_scatter(Gar[:, :], C["gvals"][:, :], posT[:, :],
                    channels=128, num_elems=BSP, num_idxs=NTS)
    Gf = lp.tile([128, BSP], i16, name="Gf")
    v.tensor_tensor_scan(Gf[:, :], Gar[:, :], Gar[:, :], 0.0, ALU.max, ALU.max)
    cnt0r = lp.tile([128, 1024], i16, name="cnt0r")
    g.local_scatter(cnt0r[:, :], Gf[:, :], U[:, :],
                    channels=128, num_elems=1024, num_idxs=BSP)
    key1p = pool.tile([128, 1024], f32, name="key1p")
    v.tensor_tensor_scan(key1p[:, ::-1], nkZ[:, ::-1], cnt0r[:, ::-1],
                         0.0, ALU.mult, ALU.add)         # backward fill = cnt0
    v.tensor_tensor(key1p[:, :], key1p[:, :], C["cfold2"][:, :], ALU.add)
    lctx.__exit__(None, None, None)

    # ================= SLOT build ===========================================
    kp1 = pool.tile([128, 16, 64], f32, name="kp1")
    k3 = key1p[:, :].rearrange("p (a b) -> p a b", b=64)
    v.tensor_tensor(kp1[:, :, 0:63], k3[:, :, 1:64], k3[:, :, 0:63], ALU.is_gt)
    v.memset(kp1[:, :, 63:64], 1.0)
    kp1f = kp1[:, :, :].rearrange("p a b -> p (a b)")
    nk1 = pool.tile([128, 1024], f32, name="nk1")
    s.activation(nk1[:, :], kp1f, AF.Copy, bias=1.0, scale=-1.0)
    idxs = pool.tile([128, 1024], i16, name="idxs")
    v._custom_dve(msub1, out=idxs[:, :], in0=kp1f, in1=key1p[:, :])
    SLOT = pool.tile([128, NTS], i16, name="SLOT")
    g.local_scatter(SLOT[:, :], C["sglob1"][:, :], idxs[:, :],
                    channels=128, num_elems=NTS, num_idxs=1024)
    v.tensor_scalar_add(SLOT[:, :], SLOT[:, :], -1.0)    # in-place: SLOT-1

    # ================= payload delivery (7 fp16 channels) ===================
    dctx = tc.tile_pool(name="dscr", bufs=1)
    dp = dctx.__enter__()
    rawtags = ["rawA", "rawB", "rawC"]

    def deliver(data_ap, name, k):
        raw = dp.tile([128, 1024], f16, name="raw_" + name, tag=rawtags[k % 3])
        g.local_scatter(raw[:, :], data_ap, SLOT[:, :],
                        channels=128, num_elems=1024, num_idxs=NTS)
        out_t = pool.tile([128, 1024], f16, name="smp_" + name)
        v.tensor_tensor_scan(out_t[:, ::-1], nk1[:, ::-1], raw[:, ::-1],
                             0.0, ALU.mult, ALU.add)
        return out_t

    Dsmp = deliver(Dch[:, :, :].rearrange("p a b -> p (a b)"), "D", 0)
    dsmp = [deliver(dch[ci][:, :, :].rearrange("p a b -> p (a b)"), f"d{ci}", 1 + ci)
            for ci in range(3)]
    ysmp = [deliver(ych[ci][:, :, :].rearrange("p a b -> p (a b)"), f"y{ci}", 4 + ci)
            for ci in range(3)]

    # ================= final math ===========================================
    val = pool.tile([128, 1024], f32, name="val")
    v.tensor_tensor(val[:, :], zt[:, :], Dsmp[:, :], ALU.subtract)
    geo = pool.tile([128, 1024], f32, name="geo")
    v.tensor_scalar(geo[:, :], val[:, :], 0.0, scalar2=None, op0=ALU.is_ge)
    v.tensor_tensor(key1p[:, :], key1p[:, :], C["cwrap"][:, :], ALU.is_gt)
    wrapm = pool.tile([128, 1024], i16, name="wrapm")
    v.tensor_tensor(wrapm[:, :], key1p[:, :], geo[:, :], ALU.mult)
    for ci in range(3):
        pt = dp.tile([128, 1024], f16, name=f"pt{ci}", tag="pt")
        yib = ych[ci][:, :, 0:1].to_broadcast([128, 16, 64])
        v.tensor_tensor(pt[:, :].rearrange("p (a b) -> p a b", b=64), yib,
                        ysmp[ci][:, :].rearrange("p (a b) -> p a b", b=64),
                        ALU.subtract)
        v.copy_predicated(dsmp[ci][:, :], wrapm[:, :], pt[:, :])
    msq = pool.tile([128, 1024], f32, name="msq")
    v._custom_dve(sq2, out=msq[:, :], in0=dsmp[0][:, :], in1=dsmp[1][:, :])
    v._custom_dve(sqa, out=msq[:, :], in0=dsmp[2][:, :], in1=msq[:, :])
    inv = pool.tile([128, 1024], f32, name="inv")
    scr = pool.tile([128, 1024], f32, name="scr_inv")
    v.reciprocal_approx_accurate(inv[:, :], msq[:, :], scr[:, :])
    s.activation(inv[:, :], inv[:, :], AF.Sqrt)          # in-place rsqrt
    v.tensor_tensor(val[:, :], val[:, :], inv[:, :], ALU.mult)  # sc in-place
    out3 = pool.tile([128, 3072], f32, name="out3")
    o3 = out3[:, :].rearrange("p (s c) -> p s c", c=3)
    for ci in range(3):
        t_ = dp.tile([128, 1024], f32, name=f"sm{ci}", tag="sm")
        v.tensor_tensor(t_[:, :], val[:, :], dsmp[ci][:, :], ALU.mult)
        v._custom_dve(aff, out=o3[:, :, ci], in0=t_[:, :], in1=ysmp[ci][:, :],
                      s0=1.0, s1=float(np.float32(cvec[ci])))
    dctx.__exit__(None, None, None)
    nc.sync.dma_start(out_dram[:, :], out3[:, :])


# ---------------------------------------------------------------------------
_BUILD_CACHE = {}


def _build(A, cvec, n_cores=8):
    key = (float(np.float32(A)), tuple(float(np.float32(x)) for x in cvec))
    if key in _BUILD_CACHE:
        return _BUILD_CACHE[key]
    nc = bacc.Bacc("TRN2", target_bir_lowering=False, debug=False,
                   num_devices=n_cores)
    x0c = nc.dram_tensor("x0c", [128, 48], f32, kind="ExternalInput")
    v0c = nc.dram_tensor("v0c", [128, 48], f32, kind="ExternalInput")
    zc = nc.dram_tensor("zc", [128, 1024], f32, kind="ExternalInput")
    cdr = {}
    for name, dt_, n in CONST_SPECS:
        cdr[name] = nc.dram_tensor("cst_" + name, [128, n], _DTMAP[dt_],
                                   kind="ExternalInput")
    Oout = nc.dram_tensor("Oout", [128, 3072], f32, kind="ExternalOutput")
    with TileContext(nc) as tc:
        with tc.tile_pool(name="pp", bufs=1) as pool:
            H = pool.tile([128, 101, 48], f32)
            build_sampling_pre = None
            build_integration(nc, tc, pool, x0c, v0c, A, cvec, H)
            build_sampling(nc, tc, pool, H, zc, cdr, cvec, Oout)
    nc.compile()
    _BUILD_CACHE[key] = nc
    return nc


def kernel(x0, v0, z_vals, ior_center, ior_amp):
    """Full inputs -> full output [16384, 64, 3] float32."""
    x0 = np.ascontiguousarray(np.asarray(x0, np.float32))
    v0 = np.ascontiguousarray(np.asarray(v0, np.float32))
    z = np.ascontiguousarray(np.asarray(z_vals, np.float32)).reshape(16384, 64)
    c = np.asarray(ior_center, np.float32).reshape(3)
    A = float(np.asarray(ior_amp, np.float32).reshape(1)[0])
    n_cores = 8
    nc = _build(A, [float(c[0]), float(c[1]), float(c[2])], n_cores)
    cst = host_consts()
    in_maps = []
    for core in range(n_cores):
        sl = slice(core * 2048, (core + 1) * 2048)
        m = {"x0c": x0[sl].reshape(128, 48).copy(),
             "v0c": v0[sl].reshape(128, 48).copy(),
             "zc": z[sl].reshape(128, 1024).copy()}
        m.update({"cst_" + k: v for k, v in cst.items()})
        in_maps.append(m)
    res = run_bass_kernel_spmd(nc, in_maps, core_ids=list(range(n_cores)))
    out = np.empty((16384, 64, 3), np.float32)
    for core in range(n_cores):
        sl = slice(core * 2048, (core + 1) * 2048)
        out[sl] = res.results[core]["Oout"].reshape(2048, 64, 3)
    return out
